# revision 2
# baseline (speedup 1.0000x reference)
"""GQA attention block (b=2, s=2048, h=2048, 16 Q heads / 4 KV heads) on 8 TRN2 cores.

Sharding: query-parallel with the K/V projection sharded 4-way per batch.
Core c handles batch c//4, query rows [512*(c%4), 512*(c%4)+512). Each core
computes K^T and V only for its OWN 512-seq chunk (1/4 of the work), then one
AllGather per replica group [[0..3],[4..7]] assembles the full 2048-seq K^T
and V on every core. Attention and the o-projection for the core's 512 query
rows follow. Outputs are disjoint row blocks; the host stitches them.

Key scheduling facts this kernel is built around (measured on this part):
- The first collective in a NEFF cannot move data until ~73us after kernel
  start (one-time CC-channel init). A tiny dependency-free dummy AllGather is
  fired at t=0 so the real K/V gather (~1MB/core, ~42us bulk) starts right at
  the init gate and completes ~125us, hiding under the Q projection.
- collective_compute BLOCKS its issuing queue (gpsimd) until the collective
  completes, and DMAs issued while a collective is pending can entangle with
  its hardware-queue semaphore accounting. So gpsimd carries only [dummy CC,
  real CC, wq g3 pushes emitted BEFORE the real CC]; all other DMA issues are
  placed so their descriptors are pushed before the CC's enter the queues.
- wq streams through a 48-buffer ring: groups 0/1 push at t=0 on the Scalar
  queue (idle pre-attention), groups 2/3 push behind a fence that waits for
  the K eviction (~28us) so they don't steal startup HBM bandwidth from the
  xt/wk/wv loads that feed the collective's critical path.
- Softmax denominator per head: DVE pairwise add tree to [128,512], ones
  matmul partition-sum+broadcast on PE, then reciprocal_approx_fast (5x
  faster than the exact DVE reciprocal; denominators are benign positive
  sums), folded into the PV eviction multiply.
- The attention phase is ACT(exp)-saturated end to end (~139us, zero stalls);
  all eviction copies run on DVE/PE so nothing but exp occupies ACT there.
- All 64 wo tiles prefetch on the Sync queue right after the gather readback,
  landing during attention so the O projection never contends for SBUF/DMA.
"""

import numpy as np
import ml_dtypes

P = 128
HID = 2048
S = 2048
QS = 512          # query rows per core
NH = 16
NKV = 4
HC = HID // P     # 16 hidden chunks
KVD = NKV * P     # 512
SCALE = 1.0 / float(np.sqrt(128.0))

_COMPILED = None


def _build():
    import concourse.bacc as bacc
    import concourse.mybir as mybir
    from concourse import bass_isa, tile
    from contextlib import ExitStack

    FP = mybir.dt.float16
    F32 = mybir.dt.float32

    nc = bacc.Bacc("TRN2", target_bir_lowering=False, debug=False,
                   num_devices=8)

    xt_d = nc.dram_tensor("xt", [HID, QS], FP, kind="ExternalInput").ap()
    wq_d = nc.dram_tensor("wq", [HID, HID], FP, kind="ExternalInput").ap()
    wk_d = nc.dram_tensor("wk", [HID, KVD], FP, kind="ExternalInput").ap()
    wv_d = nc.dram_tensor("wv", [HID, KVD], FP, kind="ExternalInput").ap()
    wo_d = nc.dram_tensor("wo", [HID, HID], FP, kind="ExternalInput").ap()
    bq_d = nc.dram_tensor("bq", [P, NH], F32, kind="ExternalInput").ap()
    bk_d = nc.dram_tensor("bk", [P, NKV], F32, kind="ExternalInput").ap()
    bv_d = nc.dram_tensor("bv", [1, KVD], FP, kind="ExternalInput").ap()
    bo_d = nc.dram_tensor("bo", [1, HID], FP, kind="ExternalInput").ap()
    out_d = nc.dram_tensor("out", [QS, HID], FP, kind="ExternalOutput").ap()

    Exp = mybir.ActivationFunctionType.Exp
    GROUPS = [[0, 1, 2, 3], [4, 5, 6, 7]]

    with tile.TileContext(nc) as tc, ExitStack() as top:
        constp = top.enter_context(tc.tile_pool(name="const", bufs=1))
        ones_r128 = constp.tile([1, P], FP, tag="ones_r128")
        nc.any.memset(ones_r128, 1.0)
        ones_sq = constp.tile([P, P], FP, tag="ones_sq")
        nc.any.memset(ones_sq, 1.0)
        bq_r = constp.tile([P, NH], F32, tag="bq_r")
        nc.sync.dma_start(out=bq_r, in_=bq_d[:, :])
        bk_r = constp.tile([P, NKV], F32, tag="bk_r")
        nc.sync.dma_start(out=bk_r, in_=bk_d[:, :])
        bv_r = constp.tile([1, KVD], FP, tag="bv_r")
        nc.sync.dma_start(out=bv_r, in_=bv_d[:, :])
        bo_r = constp.tile([1, HID], FP, tag="bo_r")
        nc.sync.dma_start(out=bo_r, in_=bo_d[:, :])

        # PE warm-up: dependency-free matmuls issued during the startup DMA
        # wait so HAM reaches full K before real work arrives.
        with ExitStack() as warm:
            wps_p = warm.enter_context(tc.tile_pool(name="wps", bufs=1,
                                                    space="PSUM"))
            wsb_p = warm.enter_context(tc.tile_pool(name="wsb", bufs=1))
            wt = wsb_p.tile([P, QS], FP, tag="wt")
            nc.vector.memset(wt, 0.0)
            wsq = wsb_p.tile([P, P], FP, tag="wsq")
            nc.vector.memset(wsq, 0.0)
            wps = wps_p.tile([P, QS], F32, tag="wps")
            for _ in range(20):
                nc.tensor.matmul(wps, wsq, wt, start=True, stop=True)

        # Long-lived per-phase outputs.
        q_p = top.enter_context(tc.tile_pool(name="q_p", bufs=1))
        k_p = top.enter_context(tc.tile_pool(name="k_p", bufs=1))
        v_p = top.enter_context(tc.tile_pool(name="v_p", bufs=1))
        o_p = top.enter_context(tc.tile_pool(name="o_p", bufs=1))
        q_sb = [q_p.tile([P, QS], FP, tag=f"q{h}", name=f"q{h}") for h in range(NH)]
        k_sb = [k_p.tile([P, S], FP, tag=f"k{g}", name=f"k{g}") for g in range(NKV)]
        v_sb = [v_p.tile([P, KVD], FP, tag=f"v{ks}", name=f"v{ks}") for ks in range(HC)]
        o_sb = [o_p.tile([P, QS], FP, tag=f"o{h}", name=f"o{h}") for h in range(NH)]

        # DRAM bounce buffers for the K/V AllGather.
        dram_p = top.enter_context(tc.tile_pool(name="dram_p", bufs=1,
                                                space="DRAM"))
        cc_in = dram_p.tile([2 * KVD, QS], FP, tag="cc_in", name="cc_in")
        cc_out = dram_p.tile([8 * KVD, QS], FP, tag="cc_out", name="cc_out")
        # Dummy warmup collective: absorbs the one-time ~70us CC-channel
        # init so the real gather below starts promptly.
        dummy_in = dram_p.tile([1, 64], FP, tag="dummy_in", name="dummy_in")
        dummy_out = dram_p.tile([4, 64], FP, tag="dummy_out", name="dummy_out")
        nc.gpsimd.collective_compute(
            "AllGather", mybir.AluOpType.bypass, replica_groups=GROUPS,
            ins=[dummy_in.opt()], outs=[dummy_out.opt()],
        )

        with ExitStack() as proj:
            xt_p = proj.enter_context(tc.tile_pool(name="xt_p", bufs=1))
            wk_p = proj.enter_context(tc.tile_pool(name="wk_p", bufs=1))
            wv_p = proj.enter_context(tc.tile_pool(name="wv_p", bufs=1))
            kvo_p = proj.enter_context(tc.tile_pool(name="kvo_p", bufs=1))
            psum_p = proj.enter_context(
                tc.tile_pool(name="psum_p", bufs=2, space="PSUM")
            )

            with ExitStack() as qph:
                wq_p = qph.enter_context(tc.tile_pool(name="wq_p", bufs=48))

                # ---- Startup DMAs. K proj needs xt+wk first; wv next; wq g0
                # streams on the gpsimd queue concurrently.
                xt_sb = []
                wk_sb = []
                wq_g0 = []
                for hc in range(HC):
                    t = xt_p.tile([P, QS], FP, tag=f"xt{hc}", name=f"xt{hc}")
                    nc.sync.dma_start(out=t, in_=xt_d[hc * P:(hc + 1) * P, :])
                    xt_sb.append(t)
                    w = wk_p.tile([P, KVD], FP, tag=f"wk{hc}", name=f"wk{hc}")
                    nc.sync.dma_start(out=w, in_=wk_d[hc * P:(hc + 1) * P, :])
                    wk_sb.append(w)
                    w2 = wq_p.tile([P, QS], FP, tag="wq", name=f"wq0_{hc}")
                    nc.scalar.dma_start(
                        out=w2, in_=wq_d[hc * P:(hc + 1) * P, 0:QS]
                    )
                    wq_g0.append(w2)
                wv_sb = []
                for hc in range(HC):
                    t = wv_p.tile([P, KVD], FP, tag=f"wv{hc}", name=f"wv{hc}")
                    nc.sync.dma_start(out=t, in_=wv_d[hc * P:(hc + 1) * P, :])
                    wv_sb.append(t)
                wq_all = {}
                for hc in range(HC):
                    wq_all[(0, hc)] = wq_g0[hc]
                for hc in range(HC):
                    w2 = wq_p.tile([P, QS], FP, tag="wq", name=f"wqp1_{hc}")
                    nc.scalar.dma_start(
                        out=w2, in_=wq_d[hc * P:(hc + 1) * P, QS:2 * QS]
                    )
                    wq_all[(1, hc)] = w2

                # ---- K^T projection (own chunk): kT[g] = (x_chunk @ wk)^T ----
                kps = [
                    psum_p.tile([P, QS], F32, tag=f"pp{j}", name=f"kps{j}")
                    for j in range(4)
                ]
                for hc in range(HC):
                    for g in range(4):
                        nc.tensor.matmul(
                            kps[g],
                            wk_sb[hc][:, g * P:(g + 1) * P],
                            xt_sb[hc],
                            start=(hc == 0),
                            stop=(hc == HC - 1),
                        )
                kT_own = []
                for g in range(4):
                    t = kvo_p.tile([P, QS], FP, tag=f"kt{g}", name=f"kT{g}")
                    nc.vector.tensor_scalar_add(t, kps[g], bk_r[:, g:g + 1])
                    kT_own.append(t)
                    nc.sync.dma_start(out=cc_in[g * P:(g + 1) * P, :], in_=t)

                # Fence: delay wq g2/g3 descriptor pushes until K proj has
                # evicted (~28us) so they don't steal startup HBM bandwidth,
                # yet still land before the collective's descriptors (~77us).
                wqfence = wq_p.tile([1, 1], FP, tag="wqf", name="wqfence")
                nc.scalar.copy(wqfence, kT_own[0][0:1, 0:1])
                for g in (2, 3):
                    eng = nc.scalar if g == 2 else nc.gpsimd
                    for hc in range(HC):
                        w2 = wq_p.tile([P, QS], FP, tag="wq", name=f"wqp{g}_{hc}")
                        eng.dma_start(
                            out=w2, in_=wq_d[hc * P:(hc + 1) * P,
                                             g * QS:(g + 1) * QS]
                        )
                        wq_all[(g, hc)] = w2

                # ---- V projection (own chunk): v = x_chunk @ wv + bv ----
                vps = [
                    psum_p.tile([P, KVD], F32, tag=f"pp{j}", name=f"vps{j}")
                    for j in range(4)
                ]
                for hc in range(HC):
                    for j in range(4):
                        nc.tensor.matmul(
                            vps[j],
                            xt_sb[hc][:, j * P:(j + 1) * P],
                            wv_sb[hc],
                            start=(hc == 0),
                            stop=False,
                        )
                v_own = []
                for j in range(4):
                    nc.tensor.matmul(vps[j], ones_r128, bv_r,
                                     start=False, stop=True)
                    t = kvo_p.tile([P, KVD], FP, tag=f"vo{j}", name=f"vown{j}")
                    nc.vector.tensor_copy(t, vps[j])
                    v_own.append(t)
                    nc.sync.dma_start(
                        out=cc_in[KVD + j * P:KVD + (j + 1) * P, :], in_=t
                    )

                # ---- AllGather K^T+V within each batch's 4 cores ----
                nc.gpsimd.collective_compute(
                    "AllGather",
                    mybir.AluOpType.bypass,
                    replica_groups=GROUPS,
                    ins=[cc_in.opt()],
                    outs=[cc_out.opt()],
                )
                # gathered K^T for group 0 first (head 0's scores), then V
                # chunks in PV-consumption order, then remaining K groups.
                for g in (0, 1):
                    for r in range(4):
                        nc.sync.dma_start(
                            out=k_sb[g][:, r * QS:(r + 1) * QS],
                            in_=cc_out[r * 2 * KVD + g * P:
                                       r * 2 * KVD + (g + 1) * P, :],
                        )
                for ks in range(HC):
                    r, j = divmod(ks, 4)
                    nc.sync.dma_start(
                        out=v_sb[ks],
                        in_=cc_out[r * 2 * KVD + KVD + j * P:
                                   r * 2 * KVD + KVD + (j + 1) * P, :],
                    )
                for g in (2, 3):
                    for r in range(4):
                        nc.sync.dma_start(
                            out=k_sb[g][:, r * QS:(r + 1) * QS],
                            in_=cc_out[r * 2 * KVD + g * P:
                                       r * 2 * KVD + (g + 1) * P, :],
                        )

                # ---- Q projection: q^T[h] = (x @ wq + bq)^T, per head ----
                # overlaps the collective; wq g1-3 stream on the Scalar queue.
                for g in range(4):
                    ps = [
                        psum_p.tile([P, QS], F32, tag=f"pp{j}", name=f"psq{g}_{j}")
                        for j in range(4)
                    ]
                    for hc in range(HC):
                        if True:
                            wq_t = wq_all[(g, hc)]
                        else:
                            wq_t = wq_p.tile([P, QS], FP, tag="wq",
                                             name=f"wq{g}_{hc}")
                            nc.scalar.dma_start(
                                out=wq_t,
                                in_=wq_d[hc * P:(hc + 1) * P, g * QS:(g + 1) * QS],
                            )
                        for j in range(4):
                            nc.tensor.matmul(
                                ps[j],
                                wq_t[:, j * P:(j + 1) * P],
                                xt_sb[hc],
                                start=(hc == 0),
                                stop=(hc == HC - 1),
                            )
                    for j in range(4):
                        h = 4 * g + j
                        nc.vector.tensor_scalar_add(q_sb[h], ps[j],
                                                    bq_r[:, h:h + 1])

        # Full wo prefetch: the proj pools just freed their SBUF, and the
        # gpsimd queue (blocked behind the collective) drains right at
        # CC-completion -- wo lands during attention, long before O proj,
        # avoiding SBUF-port contention with the O-proj matmul stream.
        wo_p = top.enter_context(tc.tile_pool(name="wo_p", bufs=1))
        wo_sb = []
        for i in range(4 * HC):
            cc, hc = divmod(i, HC)
            t = wo_p.tile([P, QS], FP, tag=f"wo_{i}", name=f"wo_{i}")
            nc.sync.dma_start(
                out=t, in_=wo_d[hc * P:(hc + 1) * P, cc * QS:(cc + 1) * QS]
            )
            wo_sb.append(t)

        # ---- Attention ----
        with ExitStack() as att:
            e_p = att.enter_context(tc.tile_pool(name="e_p", bufs=1))
            ws_p = att.enter_context(tc.tile_pool(name="ws_p", bufs=1))
            sm_p = att.enter_context(tc.tile_pool(name="sm_p", bufs=2))
            s_ps = att.enter_context(tc.tile_pool(name="s_ps", bufs=1, space="PSUM"))
            pv_ps = att.enter_context(tc.tile_pool(name="pv_ps", bufs=2, space="PSUM"))
            bc_ps = att.enter_context(tc.tile_pool(name="bc_ps", bufs=2, space="PSUM"))

            e_bufs = [e_p.tile([P, HC * QS], FP, tag=f"e{i}", name=f"ebuf{i}")
                      for i in range(2)]
            ws = ws_p.tile([P, 6144], FP, tag="ws", name="wsbuf")
            rbcs = {}
            accs = {}

            def emit_score_blk(h, blk, e_big):
                g = h // NKV
                sp = s_ps.tile([P, 1024], F32, tag=f"sp{blk % 2}",
                               name=f"s{h}_{blk}")
                for j in range(2):
                    ks = blk * 2 + j
                    nc.tensor.matmul(
                        sp[:, j * QS:(j + 1) * QS],
                        k_sb[g][:, ks * P:(ks + 1) * P],
                        q_sb[h],
                        start=True,
                        stop=True,
                    )
                nc.scalar.activation(
                    e_big[:, blk * 1024:(blk + 1) * 1024],
                    sp,
                    Exp,
                    scale=SCALE,
                )

            def emit_tree_front(h, e_big):
                nc.vector.tensor_add(ws[:, 0:2048], e_big[:, 0:2048],
                                     e_big[:, 2048:4096])

            def emit_tree_back(h, e_big):
                a = sm_p.tile([P, QS], FP, tag=f"acc{h % 2}", bufs=1,
                              name=f"acc{h}")
                nc.vector.tensor_add(ws[:, 2048:4096], e_big[:, 4096:6144],
                                     e_big[:, 6144:8192])
                nc.vector.tensor_add(ws[:, 4096:6144], ws[:, 0:2048],
                                     ws[:, 2048:4096])
                nc.vector.tensor_add(ws[:, 0:1024], ws[:, 4096:5120],
                                     ws[:, 5120:6144])
                nc.vector.tensor_add(a, ws[:, 0:512], ws[:, 512:1024])
                accs[h] = a

            def emit_denom(h):
                # partition-sum+broadcast via ones matmul, then reciprocal
                bc = bc_ps.tile([P, QS], F32, tag="bc", name=f"bc{h}")
                nc.tensor.matmul(bc, ones_sq, accs[h], start=True, stop=True)
                rbc = sm_p.tile([P, QS], F32, tag=f"rbc{h % 2}", bufs=1,
                                name=f"rbc{h}")
                nc.vector.reciprocal_approx_fast(out=rbc, in_=bc)
                rbcs[h] = rbc

            def emit_pv_blk(h, blk, e_big, pvp):
                g = h // NKV
                for j in range(2):
                    ks = blk * 2 + j
                    nc.tensor.matmul(
                        pvp,
                        v_sb[ks][:, g * P:(g + 1) * P],
                        e_big[:, ks * QS:(ks + 1) * QS],
                        start=(ks == 0),
                        stop=(ks == HC - 1),
                    )

            prev = None
            for h in range(NH):
                e_big = e_bufs[h % 2]
                pvp = None
                if prev is not None:
                    pvp = pv_ps.tile([P, QS], F32, tag="pv",
                                     name=f"pv{prev[0]}")
                for blk in range(8):
                    emit_score_blk(h, blk, e_big)
                    if blk == 2 and prev is not None:
                        emit_denom(prev[0])
                    if blk == 4:
                        emit_tree_front(h, e_big)
                    if prev is not None:
                        emit_pv_blk(prev[0], blk, prev[1], pvp)
                if prev is not None:
                    nc.vector.tensor_mul(o_sb[prev[0]], pvp, rbcs[prev[0]])
                emit_tree_back(h, e_big)
                prev = (h, e_big)

            # drain: last head's PV + denominator tail
            h = prev[0]
            pvp = pv_ps.tile([P, QS], F32, tag="pv", name=f"pv{h}")
            emit_denom(h)
            for blk in range(8):
                emit_pv_blk(h, blk, prev[1], pvp)
            nc.vector.tensor_mul(o_sb[h], pvp, rbcs[h])

        # ---- Output projection: out = o @ wo + bo ----
        with ExitStack() as oph:
            fin_p = oph.enter_context(tc.tile_pool(name="fin_p", bufs=2))
            f_ps = oph.enter_context(tc.tile_pool(name="f_ps", bufs=2, space="PSUM"))

            for cc in range(4):
                ps = [
                    f_ps.tile([P, QS], F32, tag=f"fp{sc}", name=f"psf{cc}_{sc}")
                    for sc in range(4)
                ]
                for hc in range(HC):
                    for sc in range(4):
                        nc.tensor.matmul(
                            ps[sc],
                            o_sb[hc][:, sc * P:(sc + 1) * P],
                            wo_sb[cc * HC + hc],
                            start=(hc == 0),
                            stop=False,
                        )
                for sc in range(4):
                    nc.tensor.matmul(
                        ps[sc],
                        ones_r128,
                        bo_r[:, cc * QS:(cc + 1) * QS],
                        start=False,
                        stop=True,
                    )
                    ft = fin_p.tile([P, QS], FP, tag=f"f{sc}", name=f"f{cc}_{sc}")
                    nc.scalar.copy(ft, ps[sc])
                    nc.sync.dma_start(
                        out=out_d[sc * P:(sc + 1) * P, cc * QS:(cc + 1) * QS],
                        in_=ft,
                    )

    nc.compile()
    return nc


def _get_compiled():
    global _COMPILED
    if _COMPILED is None:
        _COMPILED = _build()
    return _COMPILED


LAST_EXEC_NS = None


def kernel(x, wq, bq, wk, bk, wv, bv, wo, bo, _trace_tmpdir=None):
    from concourse.bass_utils import run_bass_kernel_spmd

    nc = _get_compiled()
    bf = np.float16

    x = np.asarray(x, np.float32)
    wq_b = np.asarray(wq, np.float32).astype(bf)
    wk_b = np.asarray(wk, np.float32).astype(bf)
    wv_b = np.asarray(wv, np.float32).astype(bf)
    wo_b = np.asarray(wo, np.float32).astype(bf)
    bq_b = np.ascontiguousarray(np.asarray(bq, np.float32).reshape(NH, P).T)
    bk_b = np.ascontiguousarray(np.asarray(bk, np.float32).reshape(NKV, P).T)
    bv_b = np.asarray(bv, np.float32).astype(bf).reshape(1, KVD)
    bo_b = np.asarray(bo, np.float32).astype(bf).reshape(1, HID)

    in_maps = []
    for c in range(8):
        b = c // 4
        qo = QS * (c % 4)
        xt_c = np.ascontiguousarray(x[b, qo:qo + QS, :].astype(bf).T)
        in_maps.append(
            {
                "xt": xt_c,
                "wq": wq_b,
                "wk": wk_b,
                "wv": wv_b,
                "wo": wo_b,
                "bq": bq_b,
                "bk": bk_b,
                "bv": bv_b,
                "bo": bo_b,
            }
        )

    kw = {}
    if _trace_tmpdir is not None:
        kw = dict(trace=True, tmpdir=_trace_tmpdir)
    res = run_bass_kernel_spmd(nc, in_maps, core_ids=list(range(8)), **kw)
    global LAST_EXEC_NS
    LAST_EXEC_NS = res.exec_time_ns

    out = np.empty((2, S, HID), np.float32)
    for c in range(8):
        b = c // 4
        qo = QS * (c % 4)
        out[b, qo:qo + QS, :] = res.results[c]["out"].astype(np.float32)
    return out
